# revision 5
# baseline (speedup 1.0000x reference)
"""Trainium2 Bass kernel for nn_AgentNet (gnn_message_passing).

Math: the reference collapses algebraically. With
  We = W_w[:, :32], Whe = W_w[:, 32:64], Whp = W_w[:, 64:66]
  e = x @ embed_w.T + embed_b            (affine in x)
  mean(e) = embed_w @ mean(x) + embed_b  (so only mean(x) [2] is global)
  z = tanh(A @ x_i + B2 @ sum(x) + c0)   A = We@embed_w [128,2]
  u = sigmoid(V @ z + V_b)
Per-core work: 125000 rows. 8-way data parallel over rows, one tiny
AllReduce of the per-shard x sums [2].

Device mapping per core:
  phase 1: dense contiguous load of x -> per-partition sums (DVE strided
           reduce) -> cross-partition sum via matmul with ones ->
           AllReduce [2] -> bias vector b = B2@s + c0 via tiny matmul.
  phase 2: groups of 4 chunks x 512 rows:
           mm_A row-tiled 4x (K=2 per 32-row group) -> PSUM [128, 4x512]
           tanh(+bias) ACT -> SBUF zT [128 units, 2048 rows]
           mm_V col-tiled 4x (M=1 at psum partitions {0,32,64,96})
           DVE copy PSUM->SBUF, DMA-gather to contiguous partition block
           of u_all; one sigmoid at the end + 2 big output DMAs.
"""

import os
import numpy as np

M_TOTAL = 1_000_000
N_CORES = 8
SHARD = M_TOTAL // N_CORES          # 125000 rows per core
CH = 512                            # rows per matmul chunk
NCHUNK_FULL = SHARD // CH           # 244 full chunks
G = NCHUNK_FULL // 4                # 61 groups of 4 lanes
TAIL = SHARD - NCHUNK_FULL * CH     # 72 rows
UMACS = (G + 31) // 32              # u_all free blocks (2)
XB = 16                             # groups per x-load macro


def _split_waits(nc, max_waits=1):
    """This walrus build rejects instructions carrying more than one sync
    wait. Move excess waits onto standalone single-wait EventSemaphore
    instructions placed just before, on the same engine (conjunction of
    waits, semantically identical)."""
    from concourse import mybir

    n = 0
    for f in nc.m.functions:
        for bb in f.blocks:
            new_insts = []
            for inst in bb.instructions:
                si = getattr(inst, "sync_info", None)
                waits = list(si.on_wait) if si is not None and si.on_wait else []
                if len(waits) > max_waits:
                    head, keep = waits[:-max_waits], waits[-max_waits:]
                    for w in head:
                        new_insts.append(
                            mybir.InstEventSemaphore(
                                name=nc.get_next_instruction_name(),
                                engine=inst.engine,
                                ins=[],
                                outs=[],
                                sync_info=mybir.SyncInfo(on_wait=[w], on_update=[]),
                            )
                        )
                        n += 1
                    si.on_wait = keep
                new_insts.append(inst)
            bb.instructions[:] = new_insts
    return n


def _build_program(vb: float):
    import concourse.bass as bass
    import concourse.tile as tile
    from concourse import mybir

    f32 = mybir.dt.float32
    AF = mybir.ActivationFunctionType

    nc = bass.Bass()
    xs = nc.declare_dram_parameter("xs", [SHARD, 2], f32, isOutput=False)
    wa = nc.declare_dram_parameter("wa", [2, 128], f32, isOutput=False)
    wb = nc.declare_dram_parameter("wb", [3, 128], f32, isOutput=False)
    wv = nc.declare_dram_parameter("wv", [128, 1], f32, isOutput=False)
    out = nc.declare_dram_parameter("out", [SHARD, 1], f32, isOutput=True)
    dbg = os.environ.get("KDEBUG") == "1"
    if dbg:
        dbg_bvec = nc.declare_dram_parameter("dbg_bvec", [128, 1], f32, isOutput=True)
        dbg_sums = nc.declare_dram_parameter("dbg_sums", [3, 1], f32, isOutput=True)
        dbg_z0 = nc.declare_dram_parameter("dbg_z0", [128, 4 * CH], f32, isOutput=True)
        dbg_uall = nc.declare_dram_parameter("dbg_uall", [128, UMACS * CH], f32, isOutput=True)

    with tile.TileContext(nc) as tc:
        with (
            tc.tile_pool(name="w", bufs=1) as wpool,
            tc.tile_pool(name="x", bufs=2) as xpool,
            tc.tile_pool(name="z", bufs=2) as zpool,
            tc.tile_pool(name="zps", bufs=1, space="PSUM") as zpspool,
            tc.tile_pool(name="ups", bufs=2, space="PSUM") as upspool,
            tc.tile_pool(name="sps", bufs=1, space="PSUM") as spspool,
            tc.tile_pool(name="dram", bufs=1, space="DRAM") as dpool,
        ):
            # ---- weights / persistent tiles ----
            wa_sb = wpool.tile([128, 128], f32)
            for t in range(4):
                nc.sync.dma_start(wa_sb[32 * t:32 * t + 2, :], wa[:, :])
            wb_sb = wpool.tile([3, 128], f32)
            nc.sync.dma_start(wb_sb[:], wb[:, :])
            wv_sb = wpool.tile([128, 1], f32)
            nc.sync.dma_start(wv_sb[:], wv[:, :])
            ones = wpool.tile([128, 1], f32)
            nc.gpsimd.memset(ones[:], 1.0)
            rhs3 = wpool.tile([3, 1], f32)
            nc.gpsimd.memset(rhs3[:], 1.0)
            bvec = wpool.tile([128, 1], f32)
            vb_sb = wpool.tile([128, 1], f32)
            nc.gpsimd.memset(vb_sb[:], vb)
            u_all = wpool.tile([128, UMACS * CH], f32)

            xs_flat = xs[:].flatten()

            # ---- phase 1: shard sums -> AllReduce -> bias vector ----
            xdense = xpool.tile([128, 2048], f32, tag="xdense")
            nc.gpsimd.memset(xdense[:], 0.0)
            n_full = (2 * SHARD) // 2048          # 122 full partitions
            rem = 2 * SHARD - n_full * 2048       # 144
            nc.sync.dma_start(
                xdense[0:n_full, :],
                xs_flat[0:n_full * 2048].rearrange("(p f) -> p f", f=2048))
            nc.sync.dma_start(
                xdense[n_full:n_full + 1, 0:rem],
                xs_flat[n_full * 2048:2 * SHARD].unsqueeze(0))
            part = xpool.tile([128, 2], f32, tag="part")
            nc.vector.tensor_reduce(
                part[:], xdense[:].rearrange("p (r q) -> p q r", q=2),
                axis=mybir.AxisListType.X, op=mybir.AluOpType.add)
            sums_ps = spspool.tile([2, 1], f32, tag="sums")
            nc.tensor.matmul(sums_ps[:], part[:], ones[:], start=True, stop=True)
            sums_sb = xpool.tile([2, 1], f32, tag="sums_sb")
            nc.vector.tensor_copy(sums_sb[:], sums_ps[:])
            cin = dpool.tile([2, 1], f32)
            cout = dpool.tile([2, 1], f32, addr_space="Shared")
            nc.sync.dma_start(cin[:], sums_sb[:])
            nc.gpsimd.collective_compute(
                "AllReduce", mybir.AluOpType.add,
                replica_groups=[list(range(N_CORES))],
                ins=[cin.opt()], outs=[cout.opt()])
            nc.sync.dma_start(rhs3[0:2, :], cout[:])
            bvec_ps = spspool.tile([128, 1], f32, tag="bvec")
            nc.tensor.matmul(bvec_ps[:], wb_sb[:], rhs3[:], start=True, stop=True)
            nc.vector.tensor_copy(bvec[:], bvec_ps[:])
            if dbg:
                nc.sync.dma_start(dbg_bvec[:, :], bvec[:])
                nc.sync.dma_start(dbg_sums[:, :], rhs3[:])

            # ---- phase 2: main loop ----
            xmac = None
            g0 = 0
            for g in range(G):
                if g % XB == 0:
                    g0 = g
                    gm_n = min(XB, G - g0)
                    xmac = xpool.tile([128, gm_n * CH], f32, tag="xmac")
                    seg = xs_flat[g0 * 4 * 2 * CH:(g0 + gm_n) * 4 * 2 * CH].rearrange(
                        "(g t j q) -> t q g j", t=4, j=CH, q=2)
                    for q in range(2):
                        for t in range(4):
                            nc.sync.dma_start(
                                xmac[32 * t + q:32 * t + q + 1, :].rearrange(
                                    "p (g j) -> p g j", j=CH),
                                seg[t, q].unsqueeze(0))
                gl = g - g0
                zpre = zpspool.tile([128, 4 * CH], f32, tag="zpre")
                for t in range(4):
                    nc.tensor.matmul(
                        zpre[:, CH * t:CH * (t + 1)],
                        wa_sb[32 * t:32 * t + 2, :],
                        xmac[32 * t:32 * t + 2, gl * CH:(gl + 1) * CH],
                        start=True, stop=True, tile_position=(32 * t, 0))
                zsb = zpool.tile([128, 4 * CH], f32, tag="zsb")
                nc.scalar.activation(zsb[:], zpre[:], AF.Tanh, bias=bvec[:, 0:1])
                if dbg and g == 0:
                    nc.sync.dma_start(dbg_z0[:, :], zsb[:])
                u_ps = upspool.tile([128, CH], f32, tag="ups")
                for t in range(4):
                    nc.tensor.matmul(
                        u_ps[32 * t:32 * t + 1, :], wv_sb[:],
                        zsb[:, CH * t:CH * (t + 1)],
                        start=True, stop=True, tile_position=(0, 32 * t))
                u_sb4 = zpool.tile([128, CH], f32, tag="usb4")
                nc.vector.tensor_copy(u_sb4[:], u_ps[:])
                m, gm = g // 32, g % 32
                nc.sync.dma_start(
                    u_all[4 * gm:4 * gm + 4, m * CH:(m + 1) * CH],
                    u_sb4[0:97:32, :])

            # ---- tail chunk (72 rows) ----
            if TAIL:
                xtail = xpool.tile([2, TAIL], f32, tag="xtail")
                nc.sync.dma_start(
                    xtail[:],
                    xs_flat[NCHUNK_FULL * 2 * CH:2 * SHARD].rearrange(
                        "(j q) -> q j", q=2))
                zpre_t = zpspool.tile([128, TAIL], f32, tag="zpre")
                nc.tensor.matmul(zpre_t[:], wa_sb[0:2, :], xtail[:],
                                 start=True, stop=True, tile_position=(0, 0))
                ztail = zpool.tile([128, TAIL], f32, tag="zsb")
                nc.scalar.activation(ztail[:], zpre_t[:], AF.Tanh, bias=bvec[:, 0:1])
                ut_ps = upspool.tile([128, TAIL], f32, tag="ups")
                nc.tensor.matmul(ut_ps[0:1, :], wv_sb[:], ztail[:],
                                 start=True, stop=True, tile_position=(0, 0))
                ut_sb = zpool.tile([1, TAIL], f32, tag="utail")
                nc.vector.tensor_copy(ut_sb[:], ut_ps[0:1, :])
                st_sb = zpool.tile([1, TAIL], f32, tag="stail")
                nc.scalar.activation(st_sb[:], ut_sb[:], AF.Sigmoid, bias=vb_sb[0:1, 0:1])
                nc.sync.dma_start(
                    out[:].flatten()[NCHUNK_FULL * CH:SHARD].unsqueeze(0), st_sb[:])

            # ---- final sigmoid + stores ----
            if dbg:
                nc.sync.dma_start(dbg_uall[:, :], u_all[:])
            usig = wpool.tile([128, UMACS * CH], f32)
            nc.scalar.activation(usig[:], u_all[:], AF.Sigmoid, bias=vb_sb[:, 0:1])
            out_flat = out[:].flatten()
            for m in range(UMACS):
                gms = min(32, G - 32 * m)
                # row = 65536*m + 2048*gm + 512*t + j  at usig[4*gm+t, m*CH+j].
                # One DMA per t: DMA APs honor a single partition dim only.
                dst4 = out_flat[65536 * m:65536 * m + 2048 * gms].rearrange(
                    "(gm t j) -> t gm j", t=4, j=CH)
                for t in range(4):
                    src = usig[t:t + 4 * (gms - 1) + 1:4, m * CH:(m + 1) * CH]
                    nc.sync.dma_start(dst4[t], src)

    _split_waits(nc)
    return nc


def kernel(state0, pt_sc, embed_w, embed_b, W_w, W_b, V_w, V_b):
    from concourse.bass_utils import run_bass_kernel_spmd

    state0 = np.asarray(state0, dtype=np.float32)
    f64 = np.float64
    We = np.asarray(W_w, f64)[:, :32]
    Whe = np.asarray(W_w, f64)[:, 32:64]
    Whp = np.asarray(W_w, f64)[:, 64:66]
    ew = np.asarray(embed_w, f64)
    eb = np.asarray(embed_b, f64)
    A = We @ ew                              # [128, 2]
    B2 = (Whe @ ew) / M_TOTAL                # [128, 2]
    c0 = We @ eb + Whe @ eb + Whp @ np.asarray(pt_sc, f64) + np.asarray(W_b, f64)
    wa_np = np.ascontiguousarray(A.T, dtype=np.float32)          # [2, 128]
    wb_np = np.ascontiguousarray(
        np.concatenate([B2, c0[:, None]], axis=1).T, dtype=np.float32)  # [3, 128]
    wv_np = np.ascontiguousarray(
        np.asarray(V_w, f64).reshape(128, 1), dtype=np.float32)
    vb = float(np.asarray(V_b).reshape(-1)[0])

    nc = _build_program(vb)

    x = state0[1:]                            # [1M, 2]
    in_maps = []
    for c in range(N_CORES):
        in_maps.append({
            "xs": np.ascontiguousarray(x[c * SHARD:(c + 1) * SHARD]),
            "wa": wa_np, "wb": wb_np, "wv": wv_np,
        })
    res = run_bass_kernel_spmd(nc, in_maps, list(range(N_CORES)))
    if res.exec_time_ns is not None:
        print(f"HW exec time: {res.exec_time_ns} ns")
    if os.environ.get("KDEBUG") == "1":
        np.savez("/root/problem/work/dbg_core0.npz", **res.results[0])
    outs = [res.results[c]["out"] for c in range(N_CORES)]
    return np.concatenate(outs, axis=0).astype(np.float32)
